# revision 2
# baseline (speedup 1.0000x reference)
"""BiMamba block on 8 Trainium2 NeuronCores — v2 (bf16, pipelined).

Sharding: d_inner (2048) split 8 ways -> 256 channels/core for in_proj/conv/
scan; out_proj is token-sharded (each core computes full d_model output for
its 512-token chunk after an AllToAll of gated SSM outputs).

vs v1:
- bf16 datapath (fp32 PSUM accum) — matmuls 2x, TT mults 1.7x faster.
- conv as 4 diagonal-weight matmuls into PSUM + Silu evac (no DVE tap ops).
- bwd branch computed/stored in FORWARD time (anti-causal conv kernel);
  time flips happen in DMA/DVE access patterns at phase-2 loads.
- per-chunk AllReduce (8 small bf16 collectives) pipelined with compute.
- AllToAll + local full out_proj replaces the big ReduceScatter.
- sz/x_pad/xc/yacc SBUF-resident; no DRAM spill.
- hlast carry via vector tensor_copy [128,1] + bf16 initial AP.
"""
import sys, os
sys.path.insert(0, '/opt/trn_rl_repo')
os.environ.setdefault("JAX_PLATFORMS", "cpu")

import numpy as np
from contextlib import ExitStack

import concourse.bass as bass
import concourse.tile as tile
from concourse import bacc, mybir
from concourse.bass_utils import run_bass_kernel_spmd

F32 = mybir.dt.float32
BF16 = mybir.dt.bfloat16
AF = mybir.ActivationFunctionType
ALU = mybir.AluOpType

B, L, DM, DI, N, R, KC = 2, 2048, 1024, 2048, 16, 64, 4
NC = 8
CH = DI // NC
TOK = B * L
CK = 512
NCK = TOK // CK
CPB = L // CK
NCB = CH // 32
NP = N // 4
PADL = L + 6

HC_ON_GP = True          # hC multiply on GpSimd (else DVE)
DB_ON_GP = False         # dB multiply on GpSimd (else DVE)

_CACHE = {}


def build_program():
    nc = bacc.Bacc("TRN2", target_bir_lowering=False, debug=False,
                   num_devices=NC)

    ext = {}
    def ein(name, shape, dt=F32):
        ext[name] = nc.dram_tensor(name, list(shape), dt,
                                   kind="ExternalInput")
        return ext[name]

    uT = ein("uT", (DM, TOK), BF16)
    winT = ein("winT", (DM, 2 * CH), BF16)
    woutT = ein("woutT", (DI, DM), BF16)
    sel = ein("sel", (128, 4 * 128), BF16)
    sel32 = ein("sel32", (128, 32), BF16)
    for p in ("f", "b"):
        ein(f"{p}dcw", (128, 8 * 128), BF16)   # diag conv mats [ct*4+k]
        ein(f"{p}cbias", (CH, 1))
        ein(f"{p}xpT", (128, 2 * 96), BF16)
        ein(f"{p}dtwT", (R, CH), BF16)
        ein(f"{p}dtb", (CH, 1))
        ein(f"{p}acols", (128, 32))
        ein(f"{p}dvec", (CH, 1))

    out_slice = nc.dram_tensor("out_slice", [DM, CK], F32,
                               kind="ExternalOutput")

    cc_in_t = [nc.dram_tensor(f"ccin{k}", [192, CK], BF16)
               for k in range(NCK)]
    cc_out_t = [nc.dram_tensor(f"ccout{k}", [192, CK], BF16,
                               addr_space="Shared") for k in range(NCK)]
    a2a_in = nc.dram_tensor("a2a_in", [DI, CK], BF16)
    a2a_out = nc.dram_tensor("a2a_out", [DI, CK], BF16)

    GROUPS = [list(range(NC))]

    with tile.TileContext(nc) as tc, ExitStack() as ctx:
        wp = ctx.enter_context(tc.tile_pool(name="wp", bufs=1))
        big = ctx.enter_context(tc.tile_pool(name="big", bufs=1))

        sel_sb = wp.tile([128, 4 * 128], BF16, name="sel_sb")
        nc.sync.dma_start(sel_sb[:], sel[:])
        sel32_sb = wp.tile([128, 32], BF16, name="sel32_sb")
        nc.sync.dma_start(sel32_sb[:], sel32[:])
        win_sb = wp.tile([128, 8 * 512], BF16, name="win_sb")
        for k in range(8):
            nc.sync.dma_start(win_sb[:, k * 512:(k + 1) * 512],
                              winT[k * 128:(k + 1) * 128, :])

        br_w = {}
        for p in ("f", "b"):
            d = {}
            d["dcw"] = wp.tile([128, 8 * 128], BF16, name=f"{p}dcw_sb")
            nc.sync.dma_start(d["dcw"][:], ext[f"{p}dcw"][:])
            for nm in ("cbias", "dtb", "dvec"):
                t_ = wp.tile([128, 2], F32, name=f"{p}{nm}_sb")
                for ct in range(2):
                    nc.sync.dma_start(
                        t_[:, ct:ct + 1],
                        ext[f"{p}{nm}"][ct * 128:(ct + 1) * 128, :])
                d[nm] = t_
            d["xpT"] = wp.tile([128, 2 * 96], BF16, name=f"{p}xpT_sb")
            nc.sync.dma_start(d["xpT"][:], ext[f"{p}xpT"][:])
            d["dtwT"] = wp.tile([R, CH], BF16, name=f"{p}dtwT_sb")
            nc.sync.dma_start(d["dtwT"][:], ext[f"{p}dtwT"][:])
            d["acols"] = wp.tile([128, 32], F32, name=f"{p}acols_sb")
            nc.sync.dma_start(d["acols"][:], ext[f"{p}acols"][:])
            br_w[p] = d

        # persistent activations (all forward-time order)
        xc = {p: [big.tile([128, TOK], BF16, name=f"xc{p}{ct}")
                  for ct in range(2)] for p in ("f", "b")}
        sz = [big.tile([128, TOK], BF16, name=f"sz{ct}") for ct in range(2)]
        yacc = [big.tile([128, TOK], BF16, name=f"yacc{ct}")
                for ct in range(2)]
        x_pad = [big.tile([128, B * PADL], BF16, name=f"xpad{ct}")
                 for ct in range(2)]
        hlast = {(p, bb): big.tile([128, 32], BF16, name=f"hl{p}{bb}")
                 for p in ("f", "b") for bb in range(B)}

        for ct in range(2):
            for bb in range(B):
                nc.vector.memset(x_pad[ct][:, bb * PADL:bb * PADL + 3], 0.0)
                nc.vector.memset(
                    x_pad[ct][:, bb * PADL + 3 + L:(bb + 1) * PADL], 0.0)

        def dcol(ckk):
            bb = ckk // CPB
            return bb * PADL + 3 + (ckk % CPB) * CK

        with tc.tile_pool(name="w1", bufs=2) as w1, \
             tc.tile_pool(name="w2", bufs=2) as w2, \
             tc.tile_pool(name="psA", bufs=2, space="PSUM") as psA, \
             tc.tile_pool(name="psB", bufs=1, space="PSUM") as psB:

            def emit_inproj(ck):
                ut = w1.tile([128, 8 * CK], BF16, tag="ut", bufs=1)
                for k in range(8):
                    nc.sync.dma_start(ut[:, k * CK:(k + 1) * CK],
                                      uT[k * 128:(k + 1) * 128,
                                         ck * CK:(ck + 1) * CK])
                for mt in range(4):
                    pin = psA.tile([128, CK], F32, tag="p1")
                    for k in range(8):
                        nc.tensor.matmul(
                            pin[:], win_sb[:, k * 512 + mt * 128:
                                           k * 512 + (mt + 1) * 128],
                            ut[:, k * CK:(k + 1) * CK],
                            start=(k == 0), stop=(k == 7))
                    if mt < 2:
                        c0 = dcol(ck)
                        nc.scalar.copy(x_pad[mt][:, c0:c0 + CK], pin[:])
                    else:
                        ct = mt - 2
                        nc.scalar.activation(
                            sz[ct][:, ck * CK:(ck + 1) * CK], pin[:],
                            AF.Silu)

            def emit_conv_xproj(p, ck):
                # conv via 4 diagonal matmuls; both branches forward-time.
                # fwd: causal taps x[c0-3+k]; bwd: anti-causal x[c0+k] with
                # host-reversed kernel. Output xc[p] stored forward.
                d = br_w[p]
                c0 = dcol(ck)
                for ct in range(2):
                    pc = psA.tile([128, CK], F32, tag="p1")
                    for k in range(4):
                        off = c0 - 3 + k if p == "f" else c0 + k
                        nc.tensor.matmul(
                            pc[:], d["dcw"][:, (ct * 4 + k) * 128:
                                            (ct * 4 + k + 1) * 128],
                            x_pad[ct][:, off:off + CK],
                            start=(k == 0), stop=(k == 3))
                    nc.scalar.activation(
                        xc[p][ct][:, ck * CK:(ck + 1) * CK], pc[:],
                        AF.Silu, bias=d["cbias"][:, ct:ct + 1])
                pxp = psB.tile([96, CK], F32, tag="pxp")
                for ct in range(2):
                    nc.tensor.matmul(
                        pxp[:], d["xpT"][:, ct * 96:(ct + 1) * 96],
                        xc[p][ct][:, ck * CK:(ck + 1) * CK],
                        start=(ct == 0), stop=(ct == 1))
                pj = w1.tile([96, CK], BF16, tag="pj")
                nc.scalar.copy(pj[:], pxp[:])
                row0 = 0 if p == "f" else 96
                nc.sync.dma_start(cc_in_t[ck][row0:row0 + 96, :], pj[:])

            def emit_ar(ck):
                nc.gpsimd.collective_compute(
                    "AllReduce", ALU.add, replica_groups=GROUPS,
                    ins=[cc_in_t[ck].ap()], outs=[cc_out_t[ck].ap()])

            # ---------- phase 1, chunk-pipelined ----------
            for bb in range(B):
                for cc in range(CPB):
                    ck = bb * CPB + cc
                    emit_inproj(ck)
                    emit_conv_xproj("f", ck)
                    if cc > 0:
                        emit_conv_xproj("b", ck - 1)
                        emit_ar(ck - 1)
                    if cc == CPB - 1:
                        emit_conv_xproj("b", ck)
                        emit_ar(ck)

            # ---------- phase 2: scans ----------
            def emit_scan_chunk(p, bb, cc):
                # p='f': forward chunk cc of batch bb, natural time order.
                # p='b': flipped-time chunk cc; forward chunk m=CPB-1-cc,
                #        loads column-reversed.
                d = br_w[p]
                rev = (p == "b")
                m = (CPB - 1 - cc) if rev else cc
                ck = bb * CPB + m          # forward chunk for data loads
                cco = cc_out_t[ck]
                row0 = 0 if p == "f" else 96

                pjc = w2.tile([R, CK], BF16, tag="pjc")
                if rev:
                    nc.sync.dma_start(pjc[:],
                                      cco[row0:row0 + R, :][:, ::-1])
                else:
                    nc.sync.dma_start(pjc[:], cco[row0:row0 + R, :])

                dts, ws, xslcs, szslcs = [], [], [], []
                for ct in range(2):
                    xslc = xc[p][ct][:, ck * CK:(ck + 1) * CK]
                    szslc = sz[ct][:, ck * CK:(ck + 1) * CK]
                    if rev:
                        xslc = xslc[:, ::-1]
                        szslc = szslc[:, ::-1]
                    xslcs.append(xslc)
                    szslcs.append(szslc)
                for ct in range(2):
                    pdt = psB.tile([128, CK], F32, tag="pdt")
                    nc.tensor.matmul(
                        pdt[:], d["dtwT"][:, ct * 128:(ct + 1) * 128],
                        pjc[:], start=True, stop=True)
                    e_ = w2.tile([128, CK], BF16, tag="edt")
                    nc.scalar.activation(e_[:], pdt[:], AF.Exp,
                                         bias=d["dtb"][:, ct:ct + 1])
                    dt_ = w2.tile([128, CK], BF16, tag=f"dt{ct}")
                    nc.scalar.activation(dt_[:], e_[:], AF.Ln, bias=1.0)
                    w_ = w2.tile([128, CK], BF16, tag=f"w{ct}")
                    nc.vector.tensor_mul(w_[:], dt_[:], xslcs[ct])
                    dts.append(dt_)
                    ws.append(w_)

                brep = w2.tile([128, NP * CK], BF16, tag="brep")
                crep = w2.tile([128, NP * CK], BF16, tag="crep")
                for np_ in range(NP):
                    for t_, base in ((brep, 64), (crep, 80)):
                        r0 = row0 + base + np_ * 4
                        ap = bass.AP(cco, r0 * CK,
                                     [[CK, 4], [0, 32], [1, CK]])
                        nc.sync.dma_start(
                            t_[:, np_ * CK:(np_ + 1) * CK], ap)

                yps = [psB.tile([128, CK], F32, tag=f"py{ct}",
                                name=f"py{ct}", bufs=1)
                       for ct in range(2)]
                for cb in range(NCB):
                    ct, j = cb // 4, cb % 4
                    pdtr = psB.tile([128, CK], F32, tag="pdtr")
                    nc.tensor.matmul(pdtr[:],
                                     sel_sb[:, j * 128:(j + 1) * 128],
                                     dts[ct][:], start=True, stop=True)
                    pwr = psB.tile([128, CK], F32, tag="pwr")
                    nc.tensor.matmul(pwr[:],
                                     sel_sb[:, j * 128:(j + 1) * 128],
                                     ws[ct][:], start=True, stop=True)
                    pw = w2.tile([128, CK], BF16, tag="pw")
                    nc.scalar.copy(pw[:], pwr[:])
                    for np_ in range(NP):
                        g = cb * NP + np_
                        bslc = brep[:, np_ * CK:(np_ + 1) * CK]
                        cslc = crep[:, np_ * CK:(np_ + 1) * CK]
                        if rev:
                            bslc = bslc[:, ::-1]
                            cslc = cslc[:, ::-1]
                        dA = w2.tile([128, CK], BF16, tag="dA", bufs=3)
                        nc.scalar.activation(
                            dA[:], pdtr[:], AF.Exp,
                            scale=d["acols"][:, g:g + 1])
                        dB = w2.tile([128, CK], BF16, tag="dB", bufs=3)
                        nc.vector.tensor_mul(dB[:], pw[:], bslc)
                        h = w2.tile([128, CK], BF16, tag="h", bufs=4)
                        init = (0.0 if cc == 0
                                else hlast[(p, bb)][:, g:g + 1])
                        nc.vector.tensor_tensor_scan(
                            h[:], dA[:], dB[:], init, ALU.mult, ALU.add)
                        if cc != CPB - 1:
                            nc.vector.tensor_copy(
                                hlast[(p, bb)][:, g:g + 1],
                                h[:, CK - 1:CK])
                        hC = w2.tile([128, CK], BF16, tag="hC", bufs=4)
                        if HC_ON_GP:
                            nc.gpsimd.tensor_mul(hC[:], h[:], cslc)
                        else:
                            nc.vector.tensor_mul(hC[:], h[:], cslc)
                        nc.tensor.matmul(
                            yps[ct][32 * j:32 * (j + 1), :],
                            sel32_sb[:], hC[:],
                            start=(np_ == 0), stop=(np_ == NP - 1),
                            tile_position=(0, 32 * j))
                # gate; yacc is forward-time indexed by ck
                for ct in range(2):
                    y1 = w2.tile([128, CK], BF16, tag="y1")
                    nc.vector.scalar_tensor_tensor(
                        y1[:], xslcs[ct], d["dvec"][:, ct:ct + 1],
                        yps[ct][:], ALU.mult, ALU.add)
                    if p == "f":
                        nc.vector.tensor_mul(
                            yacc[ct][:, ck * CK:(ck + 1) * CK], y1[:],
                            szslcs[ct])
                    else:
                        y2 = w2.tile([128, CK], BF16, tag="y2")
                        nc.vector.tensor_mul(y2[:], y1[:], szslcs[ct])
                        dst = yacc[ct][:, ck * CK:(ck + 1) * CK]
                        nc.vector.tensor_add(dst[:], dst[:], y2[:, ::-1])
                if p == "b":
                    for ct in range(2):
                        nc.sync.dma_start(
                            a2a_in[ck * 256 + ct * 128:
                                   ck * 256 + (ct + 1) * 128, :],
                            yacc[ct][:, ck * CK:(ck + 1) * CK])

            for bb in range(B):
                for cc in range(CPB):
                    emit_scan_chunk("f", bb, cc)
                for cc in range(CPB):
                    emit_scan_chunk("b", bb, cc)

            nc.gpsimd.collective_compute(
                "AllToAll", ALU.bypass, replica_groups=GROUPS,
                ins=[a2a_in.ap()], outs=[a2a_out.ap()])

        # ---------- out_proj (token-sharded, full d_model) ----------
        with tc.tile_pool(name="w3", bufs=2) as w3, \
             tc.tile_pool(name="ps3", bufs=2, space="PSUM") as ps3:
            wout_sb = w3.tile([128, 16 * DM], BF16, tag="wout", bufs=1)
            for kt in range(16):
                nc.sync.dma_start(wout_sb[:, kt * DM:(kt + 1) * DM],
                                  woutT[kt * 128:(kt + 1) * 128, :])
            ya = w3.tile([128, 16 * CK], BF16, tag="ya", bufs=1)
            for kt in range(16):
                nc.sync.dma_start(ya[:, kt * CK:(kt + 1) * CK],
                                  a2a_out[kt * 128:(kt + 1) * 128, :])
            for mt in range(8):
                po = ps3.tile([128, CK], F32, tag="po")
                for kt in range(16):
                    nc.tensor.matmul(
                        po[:], wout_sb[:, kt * DM + mt * 128:
                                       kt * DM + (mt + 1) * 128],
                        ya[:, kt * CK:(kt + 1) * CK],
                        start=(kt == 0), stop=(kt == 15))
                ob = w3.tile([128, CK], F32, tag="ob")
                nc.scalar.copy(ob[:], po[:])
                nc.sync.dma_start(
                    out_slice[mt * 128:(mt + 1) * 128, :], ob[:])

    nc.compile()
    return nc


def _prep_inputs(inputs):
    import ml_dtypes
    BF = ml_dtypes.bfloat16
    u = np.asarray(inputs["u"], np.float32)
    uT = np.ascontiguousarray(u.reshape(TOK, DM).T).astype(BF)

    sel_ = np.zeros((128, 4 * 128), np.float32)
    for j in range(4):
        for pp in range(128):
            sel_[j * 32 + pp % 32, j * 128 + pp] = 1.0
    sel32_ = np.zeros((128, 32), np.float32)
    for pp in range(128):
        sel32_[pp, pp % 32] = 1.0

    woutT = np.ascontiguousarray(
        np.asarray(inputs["out_proj_w"], np.float32).T).astype(BF)

    in_maps = []
    for core in range(NC):
        c0 = core * CH
        m = {"uT": uT, "sel": sel_.astype(BF), "sel32": sel32_.astype(BF),
             "woutT": woutT}
        W = np.asarray(inputs["in_proj_w"], np.float32)
        m["winT"] = np.ascontiguousarray(
            np.concatenate([W[c0:c0 + CH], W[DI + c0:DI + c0 + CH]],
                           0).T).astype(BF)

        for p, pref in (("f", "fwd_"), ("b", "bwd_")):
            cw = np.asarray(inputs[pref + "conv_w"],
                            np.float32)[c0:c0 + CH, 0, :]
            # diag conv mats: [ct*4+k] block is diag over 128 channels.
            # fwd uses kernel as-is (causal taps x[t-3+k]);
            # bwd anti-causal: tap x[t+k] gets weight w[3-k].
            dcw = np.zeros((128, 8 * 128), np.float32)
            for ct in range(2):
                for k in range(4):
                    blk = ct * 4 + k
                    np.fill_diagonal(
                        dcw[:, blk * 128:(blk + 1) * 128],
                        cw[ct * 128:(ct + 1) * 128,
                           k if p == "f" else 3 - k])
            m[f"{p}dcw"] = dcw.astype(BF)
            m[f"{p}cbias"] = np.ascontiguousarray(
                np.asarray(inputs[pref + "conv_b"],
                           np.float32)[c0:c0 + CH, None])
            xpT = np.asarray(inputs[pref + "x_proj_w"],
                             np.float32)[:, c0:c0 + CH].T
            xpt_pack = np.zeros((128, 2 * 96), np.float32)
            xpt_pack[:, 0:96] = xpT[0:128]
            xpt_pack[:, 96:192] = xpT[128:256]
            m[f"{p}xpT"] = xpt_pack.astype(BF)
            m[f"{p}dtwT"] = np.ascontiguousarray(
                np.asarray(inputs[pref + "dt_w"],
                           np.float32)[c0:c0 + CH].T).astype(BF)
            m[f"{p}dtb"] = np.ascontiguousarray(
                np.asarray(inputs[pref + "dt_b"],
                           np.float32)[c0:c0 + CH, None])
            A = -np.exp(np.asarray(inputs[pref + "A_log"],
                                   np.float32)[c0:c0 + CH])
            ac = np.zeros((128, 32), np.float32)
            for g in range(32):
                cb, np_ = g // NP, g % NP
                for pp in range(128):
                    ac[pp, g] = A[cb * 32 + pp % 32, np_ * 4 + pp // 32]
            m[f"{p}acols"] = ac
            m[f"{p}dvec"] = np.ascontiguousarray(
                np.asarray(inputs[pref + "D"], np.float32)[c0:c0 + CH, None])
        in_maps.append(m)
    return in_maps


def kernel(**inputs) -> np.ndarray:
    if "nc" not in _CACHE:
        _CACHE["nc"] = build_program()
    nc = _CACHE["nc"]
    in_maps = _prep_inputs(inputs)
    res = run_bass_kernel_spmd(nc, in_maps, list(range(NC)))
    out_full = np.concatenate(
        [np.asarray(res.results[i]["out_slice"]) for i in range(NC)], 1)
    y = out_full.reshape(DM, B, L).transpose(1, 2, 0)
    return np.ascontiguousarray(y).astype(np.float32)


if __name__ == "__main__":
    d = np.load('/root/problem/inputs.npz')
    inputs = {k: d[k] for k in d.files}
    got = kernel(**inputs)
    from ref_np import reference_np
    exp = reference_np(**inputs)
    err = np.abs(got - exp).max() / (np.abs(exp).max() + 1e-30)
    print("Relative error:", err)
